# revision 14
# baseline (speedup 1.0000x reference)
"""BoxFilter (9x9 box-sum, clamped borders) Trainium2 Bass kernel.

Input  x: [16, 3, 1024, 1024] f32, r=4 (hardcoded).
Output y: same shape; y[b,c,i,j] = sum of x[b,c,u,v] over the
(2r+1)x(2r+1) window centered at (i,j), clipped to the image bounds
(exactly what the reference's cumsum+diff computes).

Sharding: pure data parallel over 8 cores, 6 of the 48 images each.

The 2e-2 rel-err gate leaves huge headroom, so everything runs in
fp16 (input quantization + fp16 output give ~7e-4 rel err): input
DMA is 2 B/elem (no hi/lo split) and the output DMA is fp16 too,
upcast to f32 on the host.

Per-core structure: 9 overlapping 128-row slab POSITIONS, each
covering all 6 images (54 half-slabs).  All 6 images of a position
share ONE input DMA and ONE output DMA via 3-D access patterns
(image-stride = H*W), so the Sync/GpSimd descriptor engines issue
~10 triggers each instead of ~55 — SWDGE Q7 activity contends with
the DVE for SBUF ports, so fewer triggers also makes the scans
faster.

Each half-slab runs one of three sub-pipelines, mixed to balance the
Vector, Tensor and Scalar engines (measured per-slab costs: scan
~2.28us V / extraction ~1.11us S / matmul ~215ns per 512-col stream
T, K-independent, so the H-band contraction is free):

  A  (V-heavy): H-band matmul (2 MM) -> PSUM f32; ScalarE extracts to
     a zero-padded fp16 tile; one merged tensor_tensor_scan computes
     the 9-window running box along W (state=(y[t]+state)-y[t-9] over
     1028 steps; leading/trailing zero pads make both clamps
     automatic).
  B  (T-heavy): the full 2D box via 9 accumulating band-matmuls over
     column-shifted views of the zero-padded input slab.  ScalarE
     extraction is the final output.  No Vector work.
  B2 (T+S): box9 = box3 o box3: 3 shifted band-matmuls -> t3[m] =
     T[m-1]+T[m]+T[m+1], m in [-3,1020] (2 PSUM banks), extract fp16,
     then out[j] = t3[j-3]+t3[j]+t3[j+3] via shifted identity-matmuls
     with the last 6 cols patched by N=6 band-matmuls interleaved so
     their LDWEIGHTS hide.  Level 2 is emitted 4 half-slabs LATE so
     its dependency on the level-1 extraction never head-of-line
     blocks the strict-FIFO PE queue.
"""

import os
import numpy as np

from concourse import bass, mybir, tile, bacc
from concourse.bass_utils import run_bass_kernel_spmd

F32 = mybir.dt.float32
FP16 = mybir.dt.float16
H, W = 1024, 1024
N_CORES = 8
IPC = 6  # images per core: (16*3)/8
R = 4
D = 2 * R + 1  # 9
SEG = W + 2 * R  # 1032: per-image input segment, R zero cols each side
OSEG = W + R  # 1028: per-image output segment, data at cols [R, R+W)

# slab positions: (row0, nrows, out0, nouts, band_col)
_SLABS = (
    [(0, 128, 0, 124, 0)]
    + [(120 * i, 128, 120 * i + 4, 120, 124) for i in range(1, 8)]
    + [(960, 64, 964, 60, 244)]
)
_BAND_COLS = 304  # 124 + 120 + 60

# type grid: _TYPES[k][c] for slab position k, image c
_TYPES = [
    ["A", "A", "B2", "A", "A", "A"],
    ["B2", "A", "A", "B2", "A", "A"],
    ["A", "B2", "A", "A", "B2", "A"],
    ["B2", "A", "A", "B2", "A", "A"],
    ["A", "B2", "A", "A", "B2", "A"],
    ["B", "A", "A", "B2", "A", "A"],
    ["A", "B2", "A", "A", "B", "A"],
    ["B2", "A", "A", "B2", "A", "A"],
    ["A", "B", "A", "B2", "B", "A"],
]  # A 36, B 4, B2 14


def _band_matrix() -> np.ndarray:
    bands = np.zeros((128, _BAND_COLS), np.float16)
    for row0, nrows, out0, nouts, bc in (_SLABS[0], _SLABS[1], _SLABS[8]):
        for j in range(nouts):
            h_out = out0 + j
            lo = max(0, h_out - R) - row0
            hi = min(H - 1, h_out + R) - row0
            bands[lo : hi + 1, bc + j] = 1.0
    return bands


_CACHE: dict = {}

# Set by the most recent kernel() call (for test harnesses).
LAST_RESULTS = None


def _build():
    nc = bacc.Bacc(
        "TRN2", target_bir_lowering=False, debug=False, enable_asserts=False
    )
    x_d = nc.dram_tensor("x", [IPC, H, W], FP16, kind="ExternalInput").ap()
    bands_d = nc.dram_tensor(
        "bands", [128, _BAND_COLS], FP16, kind="ExternalInput"
    ).ap()
    ident_d = nc.dram_tensor("ident", [128, 128], FP16, kind="ExternalInput").ap()
    y_d = nc.dram_tensor("y", [IPC, H, W], FP16, kind="ExternalOutput").ap()

    ADD = mybir.AluOpType.add
    SUB = mybir.AluOpType.subtract

    XP_BUFS = 4   # [128, 6*SEG] input position tiles
    OUT_BUFS = 3  # [128, 6*OSEG] output position tiles
    YT_BUFS = 8
    T3_BUFS = 4

    with tile.TileContext(nc) as tc:
        with (
            tc.tile_pool(name="const", bufs=1) as const_pool,
            tc.tile_pool(name="xin", bufs=XP_BUFS) as in_pool,
            tc.tile_pool(name="ps2", bufs=4, space="PSUM") as ps2_pool,
            tc.tile_pool(name="yrow", bufs=YT_BUFS) as y_pool,
            tc.tile_pool(name="t3", bufs=T3_BUFS) as t3_pool,
            tc.tile_pool(name="stash", bufs=T3_BUFS) as stash_pool,
            tc.tile_pool(name="outp", bufs=OUT_BUFS) as out_pool,
        ):
            bands_t = const_pool.tile([128, _BAND_COLS], FP16)
            nc.sync.dma_start(bands_t[:], bands_d[:])
            ident_t = const_pool.tile([128, 128], FP16)
            nc.sync.dma_start(ident_t[:], ident_d[:])

            def stage2_b2(st):
                """B2 level 2: out[j] = t3[j-3] + t3[j] + t3[j+3]."""
                band_ap, stash, t3b, ob, o0, nouts, nrows = st
                ps = ps2_pool.tile([128, 1024], F32, tag="ps2")
                # ident groups cols [0,512) bank0 / [512,1018) bank1; tail
                # cols [1018,1024) bank1 via 9 direct N=6 band-MMs
                # interleaved so each tiny MM's LDWEIGHTS hides behind a
                # 512-col stream.  start=True clears has_written for the
                # WHOLE bank, so the tail group opens bank1 (start=True)
                # and the [512,1018) group uses start=False (bits already
                # clear -> first write still overwrites).
                ident_mms = [
                    (0, 512, 0, True, False), (0, 512, 3, False, False),
                    (0, 512, 6, False, True),
                    (512, 506, 0, False, False), (512, 506, 3, False, False),
                    (512, 506, 6, False, True),
                ]
                order = []
                for k in range(6):
                    order.append(("i", ident_mms[k]))
                    order.append(("t", k))
                order += [("t", k) for k in range(6, D)]
                for kind, it in order:
                    if kind == "i":
                        c0, n, s, st_, sp = it
                        nc.tensor.matmul(
                            ps[:nouts, c0 : c0 + n],
                            lhsT=ident_t[:nouts, :nouts],
                            rhs=t3b[:nouts, c0 + s : c0 + s + n],
                            start=st_,
                            stop=sp,
                            skip_group_check=True,
                        )
                    else:
                        si = it
                        nc.tensor.matmul(
                            ps[:nouts, 1018:1024],
                            lhsT=band_ap,
                            rhs=stash[:nrows, si : si + 6],
                            start=(si == 0),
                            stop=(si == D - 1),
                            skip_group_check=True,
                        )
                nc.scalar.copy(ob[:nouts, o0 + R : o0 + R + W], ps[:nouts, :])

            def emit_out_dma(ob, out0, nouts):
                nc.gpsimd.dma_start(
                    y_d[:, out0 : out0 + nouts, :].rearrange("c p w -> p c w"),
                    ob[:nouts]
                    .rearrange("p (c w) -> p c w", c=IPC)[:, :, R : R + W],
                )

            pending = []
            outs = []  # (ob, out0, nouts) per position, DMA'd one pos late
            half_idx = 0
            for ki, (row0, nrows, out0, nouts, bc) in enumerate(_SLABS):
                band_ap = bands_t[:nrows, bc : bc + nouts]

                # one input DMA for all 6 images of this position; segment
                # pads (R zero cols each side) zeroed once per pool slot
                xp = in_pool.tile([128, IPC * SEG], FP16, tag="xp")
                if ki < XP_BUFS:
                    for c in range(IPC):
                        nc.vector.memset(xp[:, c * SEG : c * SEG + R], 0.0)
                        nc.vector.memset(
                            xp[:, c * SEG + R + W : (c + 1) * SEG], 0.0
                        )
                xp3 = xp[:nrows].rearrange("p (c w) -> p c w", c=IPC)
                if ki == 0:
                    # split the first position's input per image so the
                    # PE can start after ~1/6 of the transfer
                    for c in range(IPC):
                        nc.sync.dma_start(
                            xp3[:, c : c + 1, R : R + W],
                            x_d[c : c + 1, row0 : row0 + nrows, :].rearrange(
                                "c p w -> p c w"
                            ),
                        )
                else:
                    nc.sync.dma_start(
                        xp3[:, :, R : R + W],
                        x_d[:, row0 : row0 + nrows, :].rearrange(
                            "c p w -> p c w"
                        ),
                    )

                # one output tile for all 6 images of this position
                ob = out_pool.tile([128, IPC * OSEG], FP16, tag="outp")

                for c in range(IPC):
                    typ = _TYPES[ki][c]
                    x0 = c * SEG
                    o0 = c * OSEG

                    if typ == "A":
                        ps = ps2_pool.tile([128, 1024], F32, tag="ps2")
                        for b in range(2):
                            nc.tensor.matmul(
                                ps[:nouts, b * 512 : (b + 1) * 512],
                                lhsT=band_ap,
                                rhs=xp[:nrows, x0 + R + b * 512 : x0 + R + (b + 1) * 512],
                                start=True,
                                stop=True,
                            )
                        # yt: [0:9) zeros, [9:1033) H-filtered, [1033:1037)
                        # zeros (right-border steps of the merged scan)
                        yt = y_pool.tile([128, W + D + R], FP16, tag="yrow")
                        if half_idx < YT_BUFS * 2:
                            nc.vector.memset(yt[:, 0:D], 0.0)
                            nc.vector.memset(yt[:, D + W : D + W + R], 0.0)
                        nc.scalar.copy(yt[:nouts, D : D + W], ps[:nouts, :])
                        # merged scan: state = (y[t]+state) - y[t-9]; writes
                        # box_end into ob cols [o0, o0+1028): output col j
                        # is ob[o0+R+j] (trailing zeros walk the right clamp)
                        nc.vector.tensor_tensor_scan(
                            ob[:nouts, o0 : o0 + OSEG],
                            yt[:nouts, D : D + OSEG],
                            yt[:nouts, 0:OSEG],
                            0.0,
                            op0=ADD,
                            op1=SUB,
                        )

                    elif typ == "B":
                        ps = ps2_pool.tile([128, 1024], F32, tag="ps2")
                        for b in range(2):
                            for s in range(D):
                                nc.tensor.matmul(
                                    ps[:nouts, b * 512 : (b + 1) * 512],
                                    lhsT=band_ap,
                                    rhs=xp[:nrows, x0 + s + b * 512 : x0 + s + b * 512 + 512],
                                    start=(s == 0),
                                    stop=(s == D - 1),
                                )
                        nc.scalar.copy(
                            ob[:nouts, o0 + R : o0 + R + W], ps[:nouts, :]
                        )

                    else:  # B2 level 1: t3[m], m in [-3,1020], psum col m+3
                        ps3 = ps2_pool.tile([128, 1024], F32, tag="ps2")
                        for c0 in (0, 512):
                            for s in range(3):
                                nc.tensor.matmul(
                                    ps3[:nouts, c0 : c0 + 512],
                                    lhsT=band_ap,
                                    rhs=xp[:nrows, x0 + s + c0 : x0 + s + c0 + 512],
                                    start=(s == 0),
                                    stop=(s == 2),
                                )
                        t3b = t3_pool.tile([128, 1024], FP16, tag="t3b")
                        nc.scalar.copy(t3b[:nouts, :], ps3[:nouts, :])
                        # stash the 14 input cols the deferred level-2 tail
                        # needs, so the big xp tile is released early
                        stash = stash_pool.tile([128, 14], FP16, tag="stash")
                        nc.scalar.copy(
                            stash[:nrows, :], xp[:nrows, x0 + 1018 : x0 + 1032]
                        )
                        pending.append(
                            (half_idx + 4,
                             (band_ap, stash, t3b, ob, o0, nouts, nrows))
                        )

                    while pending and pending[0][0] <= half_idx:
                        stage2_b2(pending.pop(0)[1])
                    half_idx += 1

                # the PREVIOUS position's output is complete once this
                # position's halves (and deferred B2 stage-2s) are emitted
                outs.append((ob, out0, nouts))
                if ki > 0:
                    emit_out_dma(*outs[ki - 1])

            for _, st in pending:
                stage2_b2(st)
            emit_out_dma(*outs[8])

    nc.compile()
    return nc


def kernel(x: np.ndarray, r) -> np.ndarray:
    global LAST_RESULTS
    x = np.asarray(x, dtype=np.float32)
    assert x.shape == (16, 3, H, W), x.shape
    assert int(r) == R, r

    nc = _CACHE.get("nc")
    if nc is None:
        nc = _CACHE["nc"] = _build()

    xr = x.reshape(N_CORES, IPC, H, W).astype(np.float16)
    bands = _band_matrix()
    ident = np.eye(128, dtype=np.float16)
    in_maps = [
        {"x": np.ascontiguousarray(xr[c]), "bands": bands, "ident": ident}
        for c in range(N_CORES)
    ]

    trace = bool(int(os.environ.get("BOX_TRACE", "0")))
    tmpdir = os.environ.get("BOX_TRACE_DIR") or None
    if tmpdir:
        os.makedirs(tmpdir, exist_ok=True)
    res = run_bass_kernel_spmd(
        nc, in_maps, list(range(N_CORES)), trace=trace, tmpdir=tmpdir
    )
    LAST_RESULTS = res
    y = np.stack([res.results[c]["y"] for c in range(N_CORES)])
    return y.reshape(16, 3, H, W).astype(np.float32)
